# revision 1
# baseline (speedup 1.0000x reference)
"""Trainium2 Bass kernel for nn_AV_MiltiHeadAttention.

Strategy: data-parallel over B across 8 NeuronCores (1 batch element per core).
Per-core everything is kept in a "transposed" (feature-on-partitions) layout so
no on-chip transposes are needed:

  audiaT/lipT/W*T are marshalled on host (transpose + bf16 cast = input
  layout prep for the SPMD shards), all matmul contractions then have their
  contraction dim on SBUF partitions naturally.

  S.T[m,n] = sum_d kT[d,m] qT[d,n]          (per head, K=64, 2 heads row-packed)
  E = exp(S.T * scale)                       (no max subtraction; |S*scale|<2)
  raw[j,n] = sum_m [k_h | 1][m,j] E[m,n]     -> rows 0..63 = attn_qk.T (unnorm),
                                                row 64 = softmax denominator r
  attn_qk.T = raw[0:64] * (1/r)              (partition_broadcast of 1/r)
  y = attn_qk.T * lip_v.T ; expY = exp(y)
  s = colsum_d(expY) via selector matmul     (softmax over head_dim)
  z.T = attn_qk.T * expY * (1/s)
  av_out.T = W_proj.T.T @ z.T + b_proj       ; av_attn = 0.5+0.5*tanh(qs.T@qs/temp)
  out = av_out @ av_attn                     (einsum bnc,bcd->bnd)

All matmuls bf16 x bf16 -> fp32 PSUM. Softmax normalization in fp32.
"""

import os
import numpy as np
import ml_dtypes

DEBUG_DUMP = os.environ.get("KDBG", "0") == "1"

B, N, C = 8, 1024, 512
H, HD = 8, 64
CB = C // 128          # 4 chunks of the feature dim
MB = N // 128          # 8 chunks of the token dim
NH = N // 512          # 2 halves of the token dim (matmul free dim = 512)
SCALE = HD ** -0.5
TEMP = C ** 0.5

_CACHED = {}


def build_nc():
    import concourse.bass as bass
    import concourse.tile as tile
    import concourse.mybir as mybir
    from concourse import bacc
    from contextlib import ExitStack

    f32 = mybir.dt.float32
    bf16 = mybir.dt.bfloat16
    AF = mybir.ActivationFunctionType
    MUL = mybir.AluOpType.mult

    nc = bacc.Bacc("TRN2", target_bir_lowering=False, debug=False, num_devices=B)

    d_audiaT = nc.dram_tensor("audiaT", [C, N], bf16, kind="ExternalInput")
    d_lipT = nc.dram_tensor("lipT", [C, N], bf16, kind="ExternalInput")
    d_WqkT = nc.dram_tensor("WqkT", [C, 2 * C], bf16, kind="ExternalInput")
    d_WlipT = nc.dram_tensor("WlipT", [C, C], bf16, kind="ExternalInput")
    d_WseT = nc.dram_tensor("WseT", [C, C], bf16, kind="ExternalInput")
    d_WprojT = nc.dram_tensor("WprojT", [C, C], bf16, kind="ExternalInput")
    d_bprojP = nc.dram_tensor("bprojP", [128, CB], f32, kind="ExternalInput")
    d_sel = nc.dram_tensor("sel", [128, 65], bf16, kind="ExternalInput")
    d_out = nc.dram_tensor("out", [N, C], f32, kind="ExternalOutput")
    dbg = {}
    if DEBUG_DUMP:
        for nm, shp, dt in [
            ("dbg_qT", [128, CB, N], bf16), ("dbg_kT", [128, CB, N], bf16),
            ("dbg_kaug", [128, MB, H * (HD + 1)], bf16),
            ("dbg_lipv", [128, CB, N], bf16), ("dbg_qs", [128, MB, C], bf16),
            ("dbg_E0", [128, MB, 2 * N], bf16),
            ("dbg_rawsb0", [HD + 1, N], f32), ("dbg_attn0", [128, N], bf16),
            ("dbg_rrec0", [1, N], f32), ("dbg_rrec1", [1, N], f32),
            ("dbg_y0", [128, N], bf16), ("dbg_expY0", [128, N], bf16),
            ("dbg_srl0", [1, N], f32), ("dbg_srh0", [1, N], f32),
            ("dbg_zT", [128, CB, N], bf16), ("dbg_avattn", [128, CB, C], bf16),
            ("dbg_avoutT", [128, CB, N], bf16),
        ]:
            dbg[nm] = nc.dram_tensor(nm, shp, dt, kind="ExternalOutput")

    with tile.TileContext(nc) as tc, ExitStack() as ctx:
        persist = ctx.enter_context(tc.tile_pool(name="persist", bufs=1))
        psA = ctx.enter_context(tc.tile_pool(name="psA", bufs=2, space="PSUM"))
        psB = ctx.enter_context(tc.tile_pool(name="psB", bufs=2, space="PSUM"))

        # ---- persistent SBUF tensors ----
        qT = persist.tile([128, CB, N], bf16, tag="qT")        # q.T  [(h,d), n]
        kT = persist.tile([128, CB, N], bf16, tag="kT")        # k.T  [(h,d), n]
        k_aug = persist.tile([128, MB, H * (HD + 1)], bf16, tag="k_aug")
        lip_vT = persist.tile([128, CB, N], bf16, tag="lip_vT")
        qs = persist.tile([128, MB, C], bf16, tag="qs")        # qs natural [n, c]
        WprojT_sb = persist.tile([128, CB, C], bf16, tag="WprojT_sb")
        bprojP_sb = persist.tile([128, CB], f32, tag="bprojP_sb")
        sel_sb = persist.tile([128, 65], bf16, tag="sel_sb")
        zT = persist.tile([128, CB, N], bf16, tag="zT")
        av_attn = persist.tile([128, CB, C], bf16, tag="av_attn")
        av_outT = persist.tile([128, CB, N], bf16, tag="av_outT")

        ones64 = persist.tile([1, 64], f32, tag="ones64")
        nc.vector.memset(ones64[:], 1.0)
        ones_hi = persist.tile([65, 64], f32, tag="ones_hi")
        nc.vector.memset(ones_hi[:], 1.0)

        nc.sync.dma_start(WprojT_sb[:], d_WprojT[:].rearrange("(cb p) c -> p cb c", p=128))
        nc.sync.dma_start(bprojP_sb[:], d_bprojP[:])
        nc.sync.dma_start(sel_sb[:], d_sel[:])

        early = ctx.enter_context(tc.tile_pool(name="early", bufs=1))
        audiaT_sb = early.tile([128, CB, N], bf16, tag="audiaT_sb")
        lipT_sb = early.tile([128, CB, N], bf16, tag="lipT_sb")
        WqkT_sb = early.tile([128, CB, 2 * C], bf16, tag="WqkT_sb")
        WlipT_sb = early.tile([128, CB, C], bf16, tag="WlipT_sb")
        WseT_sb = early.tile([128, CB, C], bf16, tag="WseT_sb")

        for cb in range(CB):
            nc.sync.dma_start(
                WqkT_sb[:, cb], d_WqkT[:].rearrange("(cb p) c -> cb p c", p=128)[cb]
            )
            nc.sync.dma_start(
                audiaT_sb[:, cb], d_audiaT[:].rearrange("(cb p) n -> cb p n", p=128)[cb]
            )
        nc.sync.dma_start(lipT_sb[:], d_lipT[:].rearrange("(cb p) n -> p cb n", p=128))
        nc.sync.dma_start(WlipT_sb[:], d_WlipT[:].rearrange("(cb p) c -> p cb c", p=128))
        nc.sync.dma_start(WseT_sb[:], d_WseT[:].rearrange("(cb p) c -> p cb c", p=128))

        # ---- P1 pieces (emitted interleaved with attention below) ----
        def emit_qkT():
            # qT / kT: out [oc:128, n] ; lhsT = WqkT block, rhs = audiaT
            for dst, oc0 in ((qT, 0), (kT, C)):
                for ocb in range(CB):
                    ps = psA.tile([128, N], f32, tag="psA")
                    for cb in range(CB):
                        for nh in range(NH):
                            nc.tensor.matmul(
                                ps[:, nh * 512:(nh + 1) * 512],
                                WqkT_sb[:, cb, oc0 + ocb * 128: oc0 + (ocb + 1) * 128],
                                audiaT_sb[:, cb, nh * 512:(nh + 1) * 512],
                                start=(cb == 0), stop=(cb == CB - 1),
                            )
                    nc.vector.tensor_copy(out=dst[:, ocb, :], in_=ps[:])

        def emit_kaug():
            # k natural (+ ones column) for the attn_qk.T stationary
            for mb in range(MB):
                ps = psB.tile([128, N], f32, tag="psB")
                for cb in range(CB):
                    nc.tensor.matmul(
                        ps[:, 0:512],
                        audiaT_sb[:, cb, mb * 128:(mb + 1) * 128],
                        WqkT_sb[:, cb, C:2 * C],
                        start=(cb == 0), stop=(cb == CB - 1),
                    )
                nc.vector.tensor_copy(
                    out=k_aug[:, mb].rearrange("p (h e) -> p h e", e=HD + 1)[:, :, 0:HD],
                    in_=ps[:, 0:512].rearrange("p (h d) -> p h d", d=HD),
                )
            nc.vector.memset(
                k_aug[:].rearrange("p m (h e) -> p m h e", e=HD + 1)[:, :, :, HD:HD + 1], 1.0
            )

        def emit_lipv():
            for ocb in range(CB):
                ps = psB.tile([128, N], f32, tag="psB")
                for cb in range(CB):
                    for nh in range(NH):
                        nc.tensor.matmul(
                            ps[:, nh * 512:(nh + 1) * 512],
                            WlipT_sb[:, cb, ocb * 128:(ocb + 1) * 128],
                            lipT_sb[:, cb, nh * 512:(nh + 1) * 512],
                            start=(cb == 0), stop=(cb == CB - 1),
                        )
                nc.vector.tensor_copy(out=lip_vT[:, ocb, :], in_=ps[:])

        def emit_qs():
            # qs natural [n, c] (needed on both sides of the SE bmm)
            for nb in range(MB):
                ps = psB.tile([128, N], f32, tag="psB")
                for cb in range(CB):
                    nc.tensor.matmul(
                        ps[:, 0:512],
                        audiaT_sb[:, cb, nb * 128:(nb + 1) * 128],
                        WseT_sb[:, cb, :],
                        start=(cb == 0), stop=(cb == CB - 1),
                    )
                nc.vector.tensor_copy(out=qs[:, nb, :], in_=ps[:, 0:512])

        # ---- attention + SE ----
        attn_ctx = tc.tile_pool(name="attn", bufs=1)
        attn = attn_ctx.__enter__()

        def emit_se():
            # x[c,d] = sum_n qs[n,c] qs[n,d];  av_attn = 0.5 + 0.5*tanh(x/temp)
            for cb in range(CB):
                ps = psB.tile([128, N], f32, tag="psB")
                for nb in range(MB):
                    nc.tensor.matmul(
                        ps[:, 0:512],
                        qs[:, nb, cb * 128:(cb + 1) * 128],
                        qs[:, nb, :],
                        start=(nb == 0), stop=(nb == MB - 1),
                    )
                nc.scalar.activation(av_attn[:, cb, :], ps[:, 0:512], AF.Tanh, scale=1.0 / TEMP)
                nc.vector.tensor_scalar(
                    av_attn[:, cb, :], av_attn[:, cb, :], 0.5, 0.5,
                    mybir.AluOpType.mult, mybir.AluOpType.add,
                )

        def emit_S(p):
            # E = exp(S.T * scale) for heads (2p, 2p+1)
            E = attn.tile([128, MB, 2 * N], bf16, tag="E", bufs=2, name=f"E{p}")
            for mb in range(MB):
                for hh in range(2):
                    ps = psA.tile([128, N], f32, tag="psA")
                    base = hh * 64
                    for nh in range(NH):
                        nc.tensor.matmul(
                            ps[:, nh * 512:(nh + 1) * 512],
                            kT[base:base + 64, p, mb * 128:(mb + 1) * 128],
                            qT[base:base + 64, p, nh * 512:(nh + 1) * 512],
                            start=True, stop=True,
                            tile_position=(base, 0),
                        )
                    nc.scalar.activation(
                        E[:, mb, hh * N:(hh + 1) * N], ps[:], AF.Exp, scale=SCALE
                    )
            return E

        def emit_qk(p, E):
            # raw = [k_h|1].T @ E_h for BOTH heads back-to-back (keeps the PE
            # dense so HAM stays at full clock), then the normalize chains.
            attn_sb = attn.tile([128, N], bf16, tag="attn_sb", bufs=2, name=f"attn_sb{p}")
            raws = []
            for hh in range(2):
                h = 2 * p + hh
                raw = psB.tile([128, N], f32, tag="psB")
                for mb in range(MB):
                    for nh in range(NH):
                        nc.tensor.matmul(
                            raw[0:HD + 1, nh * 512:(nh + 1) * 512],
                            k_aug[:, mb, h * (HD + 1):(h + 1) * (HD + 1)],
                            E[:, mb, hh * N + nh * 512: hh * N + (nh + 1) * 512],
                            start=(mb == 0), stop=(mb == MB - 1),
                        )
                # single psum->sbuf copy of rows 0..64: rows 0..63 feed the
                # normalize TT, row 64 (= r) feeds the reciprocal (the custom
                # DVE reciprocal reads garbage from PSUM on hw, so via SBUF).
                raw_sb = attn.tile([HD + 1, N], f32, tag="raw_sb", bufs=2, name=f"raw_sb{p}_{hh}")
                nc.vector.tensor_copy(out=raw_sb[:], in_=raw[0:HD + 1, :])
                raws.append(raw_sb)
            for hh in range(2):
                base = hh * 64
                raw_sb = raws[hh]
                # broadcast the raw r row across partitions with a K=1
                # matmul (base-64 rhs, hw-verified), stage to SBUF, then take
                # the reciprocal at base partition 0 (custom-DVE recip only
                # works from partition-0 SBUF on hw).
                rbraw = psB.tile([128, N], f32, tag="psB")
                for nh in range(NH):
                    nc.tensor.matmul(
                        rbraw[0:64, nh * 512:(nh + 1) * 512],
                        ones_hi[HD:HD + 1, :],
                        raw_sb[HD:HD + 1, nh * 512:(nh + 1) * 512],
                        start=True, stop=True,
                        tile_position=(64, 0),
                    )
                rcop = attn.tile([64, N], f32, tag="r_row", bufs=2, name=f"rcop{p}_{hh}")
                nc.vector.tensor_copy(out=rcop[:], in_=rbraw[0:64, :])
                rinv = attn.tile([64, N], f32, tag="r_rec", bufs=2, name=f"rinv{p}_{hh}")
                nc.vector.reciprocal_approx_fast(out=rinv[:], in_=rcop[:])
                nc.vector.tensor_tensor(
                    attn_sb[base:base + 64, :], raw_sb[0:HD, :], rinv[:], MUL
                )

            # second softmax (over head_dim) + z.T
            y = attn.tile([128, N], bf16, tag="y", bufs=2, name=f"y{p}")
            nc.vector.tensor_tensor(y[:], attn_sb[:], lip_vT[:, p, :], MUL)
            expY = attn.tile([128, N], bf16, tag="expY", bufs=2, name=f"expY{p}")
            nc.scalar.activation(expY[:], y[:], AF.Exp)

            s_ps = psB.tile([128, N], f32, tag="psB")
            for nh in range(NH):
                nc.tensor.matmul(
                    s_ps[0:65, nh * 512:(nh + 1) * 512],
                    sel_sb[:],
                    expY[:, nh * 512:(nh + 1) * 512],
                    start=True, stop=True,
                )
            s_sb = attn.tile([HD + 1, N], f32, tag="raw_sb", bufs=2, name=f"s_sb{p}")
            nc.vector.tensor_copy(out=s_sb[:], in_=s_ps[0:HD + 1, :])
            s_hi_row = attn.tile([1, N], f32, tag="r_row", bufs=2, name=f"shr{p}")
            nc.sync.dma_start(s_hi_row[:], s_sb[HD:HD + 1, :])
            s_rec_lo = attn.tile([1, N], f32, tag="r_rec", bufs=2, name=f"srl{p}")
            s_rec_hi = attn.tile([1, N], f32, tag="r_rec", bufs=2, name=f"srh{p}")
            nc.vector.reciprocal_approx_fast(out=s_rec_lo[:], in_=s_sb[0:1, :])
            nc.vector.reciprocal_approx_fast(out=s_rec_hi[:], in_=s_hi_row[:])
            sb_lo = psB.tile([128, N], f32, tag="psB")
            sb_hi = psB.tile([128, N], f32, tag="psB")
            for nh in range(NH):
                nc.tensor.matmul(
                    sb_lo[0:64, nh * 512:(nh + 1) * 512], ones64[:],
                    s_rec_lo[:, nh * 512:(nh + 1) * 512], start=True, stop=True,
                )
                nc.tensor.matmul(
                    sb_hi[0:64, nh * 512:(nh + 1) * 512], ones64[:],
                    s_rec_hi[:, nh * 512:(nh + 1) * 512], start=True, stop=True,
                )

            u = attn.tile([128, N], bf16, tag="y", bufs=2, name=f"u{p}")
            nc.vector.tensor_tensor(u[:], attn_sb[:], expY[:], MUL)
            nc.vector.tensor_tensor(zT[0:64, p, :], u[0:64, :], sb_lo[0:64, :], MUL)
            nc.vector.tensor_tensor(zT[64:128, p, :], u[64:128, :], sb_hi[0:64, :], MUL)
            if DEBUG_DUMP and p == 0:
                nc.sync.dma_start(dbg["dbg_attn0"][:], attn_sb[:])
                nc.sync.dma_start(dbg["dbg_y0"][:], y[:])
                nc.sync.dma_start(dbg["dbg_expY0"][:], expY[:])
                nc.sync.dma_start(dbg["dbg_srl0"][:], s_rec_lo[:])
                nc.sync.dma_start(dbg["dbg_srh0"][:], s_rec_hi[:])

        # 2-deep software pipeline: S(p+1) overlaps qk(p); the independent
        # projection/SE matmuls are spread through the ACT-bound S windows
        # as PE filler.
        emit_qkT()
        E0 = emit_S(0)
        emit_kaug()
        E1 = emit_S(1)
        emit_lipv()
        emit_qk(0, E0)
        if DEBUG_DUMP:
            nc.sync.dma_start(dbg["dbg_E0"][:], E0[:])
        E2 = emit_S(2)
        emit_qs()
        emit_qk(1, E1)
        E3 = emit_S(3)
        emit_qk(2, E2)
        emit_se()
        emit_qk(3, E3)
        attn_ctx.__exit__(None, None, None)
        if DEBUG_DUMP:
            nc.sync.dma_start(dbg["dbg_qT"][:], qT[:])
            nc.sync.dma_start(dbg["dbg_kT"][:], kT[:])
            nc.sync.dma_start(dbg["dbg_kaug"][:], k_aug[:])
            nc.sync.dma_start(dbg["dbg_lipv"][:], lip_vT[:])
            nc.sync.dma_start(dbg["dbg_qs"][:], qs[:])
            nc.sync.dma_start(dbg["dbg_zT"][:], zT[:])
            nc.sync.dma_start(dbg["dbg_avattn"][:], av_attn[:])

        # ---- P4: av_out.T = W_proj.T.T @ z.T + b_proj ----
        for ocb in range(CB):
            ps = psA.tile([128, N], f32, tag="psA")
            for cb in range(CB):
                for nh in range(NH):
                    nc.tensor.matmul(
                        ps[:, nh * 512:(nh + 1) * 512],
                        WprojT_sb[:, cb, ocb * 128:(ocb + 1) * 128],
                        zT[:, cb, nh * 512:(nh + 1) * 512],
                        start=(cb == 0), stop=(cb == CB - 1),
                    )
            nc.vector.tensor_scalar(
                av_outT[:, ocb, :], ps[:], bprojP_sb[:, ocb:ocb + 1], None,
                mybir.AluOpType.add,
            )

        # ---- P5: out[n,d] = sum_c av_out[n,c] av_attn[c,d] ----
        with tc.tile_pool(name="outp", bufs=1) as outp:
            for nb in range(MB):
                ps = psA.tile([128, N], f32, tag="psA")
                for cb in range(CB):
                    nc.tensor.matmul(
                        ps[:, 0:512],
                        av_outT[:, cb, nb * 128:(nb + 1) * 128],
                        av_attn[:, cb, :],
                        start=(cb == 0), stop=(cb == CB - 1),
                    )
                o_sb = outp.tile([128, C], f32, tag="o_sb")
                nc.vector.tensor_copy(out=o_sb[:], in_=ps[:, 0:512])
                nc.sync.dma_start(
                    d_out[:].rearrange("(nb p) c -> nb p c", p=128)[nb], o_sb[:]
                )

    nc.compile()
    return nc


def _marshal(audia, lip, W_qkv, W_lip, W_proj, b_proj, W_se):
    bf16 = ml_dtypes.bfloat16
    WqkT = np.ascontiguousarray(W_qkv[:2 * C].T.astype(bf16))
    WlipT = np.ascontiguousarray(W_lip.T.astype(bf16))
    WseT = np.ascontiguousarray(W_se.T.astype(bf16))
    WprojT = np.ascontiguousarray(W_proj.T.astype(bf16))
    bprojP = np.ascontiguousarray(
        np.asarray(b_proj, np.float32).reshape(CB, 128).T
    )
    sel = np.zeros((128, 65), bf16)
    sel[0:64, 0] = 1
    sel[64:128, 64] = 1
    in_maps = []
    for b in range(B):
        in_maps.append({
            "audiaT": np.ascontiguousarray(audia[b].T.astype(bf16)),
            "lipT": np.ascontiguousarray(lip[b].T.astype(bf16)),
            "WqkT": WqkT, "WlipT": WlipT, "WseT": WseT, "WprojT": WprojT,
            "bprojP": bprojP, "sel": sel,
        })
    return in_maps


def run(inputs, trace=False, **kw):
    from concourse.bass_utils import run_bass_kernel_spmd
    if "nc" not in _CACHED:
        _CACHED["nc"] = build_nc()
    in_maps = _marshal(**inputs)
    return run_bass_kernel_spmd(
        _CACHED["nc"], in_maps, core_ids=list(range(B)), trace=trace, **kw
    )


def kernel(audia, lip, W_qkv, W_lip, W_proj, b_proj, W_se):
    res = run(dict(audia=audia, lip=lip, W_qkv=W_qkv, W_lip=W_lip,
                   W_proj=W_proj, b_proj=b_proj, W_se=W_se))
    return np.stack([r["out"] for r in res.results], 0).astype(np.float32)



# revision 20
# speedup vs baseline: 1.2482x; 1.2482x over previous
"""Trainium2 Bass kernel for nn_AV_MiltiHeadAttention.

Strategy: data-parallel over B across 8 NeuronCores (1 batch element per core).
Per-core everything is kept in a "transposed" (feature-on-partitions) layout so
no on-chip transposes are needed:

  audiaT/lipT/W*T are marshalled on host (transpose + bf16 cast = input
  layout prep for the SPMD shards), all matmul contractions then have their
  contraction dim on SBUF partitions naturally.

  S.T[m,n] = sum_d kT[d,m] qT[d,n]          (per head, K=64, 2 heads row-packed)
  E = exp(S.T * scale)                       (no max subtraction; |S*scale|<2)
  raw[j,n] = sum_m [1 | k_h][m,j] E[m,n]    -> row 0 = softmax denominator r,
                                               rows 1..64 = attn_qk.T (unnorm)
  rinv = 1/r (DVE recip on [1,N]); broadcast across 64 partitions on GpSimd
  attn_qk.T = raw[1:65] * rinv_b
  y = attn_qk.T * lip_v.T ; expY = exp(y)
  s = colsum_d(expY) via 2-col selector matmul (softmax over head_dim)
  z.T = attn_qk.T * expY * (1/s)             (1/s broadcast on GpSimd)
  av_attn = sigmoid(2 * qs.T@qs / temp)      (SE layer)
  Wc = W_proj.T @ av_attn ; row = b_proj @ av_attn   (proj/out fusion)
  out = z @ Wc + row                          (single fused output GEMM)

All matmuls bf16 x bf16 -> fp32 PSUM. Softmax normalization in fp32.
"""

import os
import numpy as np
import ml_dtypes

DEBUG_DUMP = os.environ.get("KDBG", "0") == "1"

B, N, C = 8, 1024, 512
H, HD = 8, 64
CB = C // 128          # 4 chunks of the feature dim
MB = N // 128          # 8 chunks of the token dim
NH = N // 512          # 2 halves of the token dim (matmul free dim = 512)
SCALE = HD ** -0.5
TEMP = C ** 0.5

_CACHED = {}


def build_nc():
    import concourse.bass as bass
    import concourse.tile as tile
    import concourse.mybir as mybir
    from concourse import bacc
    from concourse import library_config
    from contextlib import ExitStack

    f32 = mybir.dt.float32
    bf16 = mybir.dt.bfloat16
    AF = mybir.ActivationFunctionType
    MUL = mybir.AluOpType.mult
    ADD = mybir.AluOpType.add

    nc = bacc.Bacc("TRN2", target_bir_lowering=False, debug=False, num_devices=B)

    d_audiaT = nc.dram_tensor("audiaT", [C, N], bf16, kind="ExternalInput")
    d_lipT = nc.dram_tensor("lipT", [C, N], bf16, kind="ExternalInput")
    d_WqkT = nc.dram_tensor("WqkT", [C, 2 * C], bf16, kind="ExternalInput")
    d_WlipT = nc.dram_tensor("WlipT", [C, C], bf16, kind="ExternalInput")
    d_WseT = nc.dram_tensor("WseT", [C, C], bf16, kind="ExternalInput")
    d_WprojN = nc.dram_tensor("WprojN", [C, C], bf16, kind="ExternalInput")
    d_bprojB = nc.dram_tensor("bprojB", [128, CB], bf16, kind="ExternalInput")
    d_sel = nc.dram_tensor("sel", [128, 2], bf16, kind="ExternalInput")
    d_selB = nc.dram_tensor("selB", [2, 128], bf16, kind="ExternalInput")
    d_out = nc.dram_tensor("out", [N, C], f32, kind="ExternalOutput")
    dbg = {}
    if DEBUG_DUMP:
        for nm, shp, dt in [
            ("dbg_qT", [128, CB, N], bf16), ("dbg_kT", [128, CB, N], bf16),
            ("dbg_kaug", [128, MB, H * (HD + 1)], bf16),
            ("dbg_lipv", [128, CB, N], bf16), ("dbg_qs", [128, MB, C], bf16),
            ("dbg_E0", [128, MB, 2 * N], bf16),
            ("dbg_rawsb0", [HD + 1, N], f32), ("dbg_attn0", [128, N], bf16),
            ("dbg_rinvb0", [HD, N], f32),
            ("dbg_y0", [128, N], bf16), ("dbg_expY0", [128, N], bf16),
            ("dbg_srec0", [2, N], f32),
            ("dbg_sblo0", [HD, N], f32), ("dbg_sbhi0", [HD, N], f32),
            ("dbg_zT", [128, CB, N], bf16), ("dbg_avattn", [128, CB, C], bf16),
            ("dbg_wc", [128, CB, C], bf16), ("dbg_rowb", [128, C], f32),
        ]:
            dbg[nm] = nc.dram_tensor(nm, shp, dt, kind="ExternalOutput")

    with tile.TileContext(nc) as tc, ExitStack() as ctx:
        persist = ctx.enter_context(tc.tile_pool(name="persist", bufs=1))
        psA = ctx.enter_context(tc.tile_pool(name="psA", bufs=2, space="PSUM"))
        psB = ctx.enter_context(tc.tile_pool(name="psB", bufs=2, space="PSUM"))

        nc.gpsimd.load_library(library_config.attn)

        # ---- persistent SBUF tensors ----
        qT = persist.tile([128, CB, N], bf16, tag="qT")        # q.T  [(h,d), n]
        kT = persist.tile([128, CB, N], bf16, tag="kT")        # k.T  [(h,d), n]
        k_aug = persist.tile([128, MB, H * (HD + 1)], bf16, tag="k_aug")
        lip_vT = persist.tile([128, CB, N], bf16, tag="lip_vT")
        qs = persist.tile([128, MB, C], bf16, tag="qs")        # qs natural [n, c]
        WprojN_sb = persist.tile([128, CB, C], bf16, tag="WprojN_sb")
        Wc_sb = persist.tile([128, CB, C], bf16, tag="Wc_sb")
        bprojB_sb = persist.tile([128, CB], bf16, tag="bprojB_sb")
        sel_sb = persist.tile([128, 2], bf16, tag="sel_sb")
        zT = persist.tile([128, CB, N], bf16, tag="zT")
        av_attn = persist.tile([128, CB, C], bf16, tag="av_attn")
        rowb = persist.tile([128, C], f32, tag="rowb")
        # K=2 selector stationaries for the 1/s partition broadcast matmuls:
        # selB[:, 0:64] picks row 0 (s_lo), selB[:, 64:128] picks row 1 (s_hi)
        selB = persist.tile([2, 128], bf16, tag="selB")

        early = ctx.enter_context(tc.tile_pool(name="early", bufs=1))
        audiaT_sb = early.tile([128, CB, N], bf16, tag="audiaT_sb")
        lipT_sb = early.tile([128, CB, N], bf16, tag="lipT_sb")
        WqkT_sb = early.tile([128, CB, 2 * C], bf16, tag="WqkT_sb")
        WlipT_sb = early.tile([128, CB, C], bf16, tag="WlipT_sb")
        WseT_sb = early.tile([128, CB, C], bf16, tag="WseT_sb")

        # input DMAs ordered by first use: qkT needs WqkT(q half) + audiaT
        # first; the k half feeds emit_kaug; lip/se/proj weights follow.
        d_WqkT_r = d_WqkT[:].rearrange("(cb p) c -> cb p c", p=128)
        d_audiaT_r = d_audiaT[:].rearrange("(cb p) n -> cb p n", p=128)
        for cb in range(CB):
            nc.sync.dma_start(WqkT_sb[:, cb, 0:C], d_WqkT_r[cb][:, 0:C])
            nc.sync.dma_start(audiaT_sb[:, cb], d_audiaT_r[cb])
        for cb in range(CB):
            nc.sync.dma_start(WqkT_sb[:, cb, C:2 * C], d_WqkT_r[cb][:, C:2 * C])
        nc.sync.dma_start(WlipT_sb[:], d_WlipT[:].rearrange("(cb p) c -> p cb c", p=128))
        nc.sync.dma_start(lipT_sb[:], d_lipT[:].rearrange("(cb p) n -> p cb n", p=128))
        nc.sync.dma_start(sel_sb[:], d_sel[:])
        nc.sync.dma_start(selB[:], d_selB[:])
        nc.sync.dma_start(WseT_sb[:], d_WseT[:].rearrange("(cb p) c -> p cb c", p=128))
        nc.sync.dma_start(WprojN_sb[:], d_WprojN[:].rearrange("(cb p) c -> p cb c", p=128))
        nc.sync.dma_start(bprojB_sb[:], d_bprojB[:])

        # ---- P1 pieces (emitted interleaved with attention below) ----
        def emit_qkT():
            # qT / kT: out [oc:128, n] ; lhsT = WqkT block, rhs = audiaT
            for dst, oc0 in ((qT, 0), (kT, C)):
                for ocb in range(CB):
                    ps = psA.tile([128, N], f32, tag="psA")
                    for cb in range(CB):
                        for nh in range(NH):
                            nc.tensor.matmul(
                                ps[:, nh * 512:(nh + 1) * 512],
                                WqkT_sb[:, cb, oc0 + ocb * 128: oc0 + (ocb + 1) * 128],
                                audiaT_sb[:, cb, nh * 512:(nh + 1) * 512],
                                start=(cb == 0), stop=(cb == CB - 1),
                            )
                    nc.vector.tensor_copy(out=dst[:, ocb, :], in_=ps[:])

        def emit_kaug():
            # k natural (+ trailing ones column: raw row 64 = softmax denom r)
            for mb in range(MB):
                ps = psB.tile([128, N], f32, tag="psB")
                for cb in range(CB):
                    nc.tensor.matmul(
                        ps[:, 0:512],
                        audiaT_sb[:, cb, mb * 128:(mb + 1) * 128],
                        WqkT_sb[:, cb, C:2 * C],
                        start=(cb == 0), stop=(cb == CB - 1),
                    )
                nc.vector.tensor_copy(
                    out=k_aug[:, mb].rearrange("p (h e) -> p h e", e=HD + 1)[:, :, 0:HD],
                    in_=ps[:, 0:512].rearrange("p (h d) -> p h d", d=HD),
                )
            nc.vector.memset(
                k_aug[:].rearrange("p m (h e) -> p m h e", e=HD + 1)[:, :, :, HD:HD + 1], 1.0
            )

        def emit_lipv():
            for ocb in range(CB):
                ps = psB.tile([128, N], f32, tag="psB")
                for cb in range(CB):
                    for nh in range(NH):
                        nc.tensor.matmul(
                            ps[:, nh * 512:(nh + 1) * 512],
                            WlipT_sb[:, cb, ocb * 128:(ocb + 1) * 128],
                            lipT_sb[:, cb, nh * 512:(nh + 1) * 512],
                            start=(cb == 0), stop=(cb == CB - 1),
                        )
                nc.vector.tensor_copy(out=lip_vT[:, ocb, :], in_=ps[:])

        def emit_qs():
            # qs natural [n, c] (needed on both sides of the SE bmm)
            for nb in range(MB):
                ps = psB.tile([128, N], f32, tag="psB")
                for cb in range(CB):
                    nc.tensor.matmul(
                        ps[:, 0:512],
                        audiaT_sb[:, cb, nb * 128:(nb + 1) * 128],
                        WseT_sb[:, cb, :],
                        start=(cb == 0), stop=(cb == CB - 1),
                    )
                nc.vector.tensor_copy(out=qs[:, nb, :], in_=ps[:, 0:512])

        # ---- attention + SE ----
        attn_ctx = tc.tile_pool(name="attn", bufs=1)
        attn = attn_ctx.__enter__()

        def emit_se():
            # x[c,d] = sum_n qs[n,c] qs[n,d];  av_attn = sigmoid(2x/temp)
            for cb in range(CB):
                ps = psB.tile([128, N], f32, tag="psB")
                for nb in range(MB):
                    nc.tensor.matmul(
                        ps[:, 0:512],
                        qs[:, nb, cb * 128:(cb + 1) * 128],
                        qs[:, nb, :],
                        start=(nb == 0), stop=(nb == MB - 1),
                    )
                nc.scalar.activation(av_attn[:, cb, :], ps[:, 0:512], AF.Sigmoid,
                                     scale=2.0 / TEMP)

        def emit_wc():
            # Wc = W_proj.T @ av_attn  (fold the proj GEMM into the output GEMM)
            for ccb in range(CB):
                ps = psA.tile([128, N], f32, tag="psA")
                for eb in range(CB):
                    nc.tensor.matmul(
                        ps[:, 0:512],
                        WprojN_sb[:, eb, ccb * 128:(ccb + 1) * 128],
                        av_attn[:, eb, :],
                        start=(eb == 0), stop=(eb == CB - 1),
                    )
                nc.scalar.activation(Wc_sb[:, ccb, :], ps[:, 0:512], AF.Copy)
            # row = b_proj @ av_attn, broadcast across partitions for the
            # final bias add (fused into the PSUM->SBUF evacuation).
            rp = psB.tile([128, N], f32, tag="psB")
            for cb in range(CB):
                nc.tensor.matmul(
                    rp[0:1, 0:512],
                    bprojB_sb[:, cb:cb + 1],
                    av_attn[:, cb, :],
                    start=(cb == 0), stop=(cb == CB - 1),
                )
            row_sb = persist.tile([1, C], f32, tag="row_sb")
            nc.vector.tensor_copy(out=row_sb[:], in_=rp[0:1, 0:512])
            nc.gpsimd.partition_broadcast(rowb[:], row_sb[:], channels=128)
            if DEBUG_DUMP:
                nc.sync.dma_start(dbg["dbg_wc"][:], Wc_sb[:])
                nc.sync.dma_start(dbg["dbg_rowb"][:], rowb[:])

        def emit_S(p):
            # E = exp(S.T * scale) for heads (2p, 2p+1)
            E = attn.tile([128, MB, 2 * N], bf16, tag="E", bufs=2, name=f"E{p}")
            for mb in range(MB):
                for hh in range(2):
                    ps = psA.tile([128, N], f32, tag="psA")
                    base = hh * 64
                    for nh in range(NH):
                        nc.tensor.matmul(
                            ps[:, nh * 512:(nh + 1) * 512],
                            kT[base:base + 64, p, mb * 128:(mb + 1) * 128],
                            qT[base:base + 64, p, nh * 512:(nh + 1) * 512],
                            start=True, stop=True,
                            tile_position=(base, 0),
                        )
                    nc.scalar.activation(
                        E[:, mb, hh * N:(hh + 1) * N], ps[:], AF.Exp, scale=SCALE
                    )
            return E

        def emit_qk(p, E):
            # raw = [1|k_h].T @ E_h for BOTH heads back-to-back (keeps the PE
            # dense so HAM stays at full clock), then the normalize chains.
            attn_sb = attn.tile([128, N], bf16, tag="attn_sb", bufs=2, name=f"attn_sb{p}")
            raws = []
            for hh in range(2):
                h = 2 * p + hh
                raw = psB.tile([128, N], f32, tag="psB")
                for mb in range(MB):
                    for nh in range(NH):
                        nc.tensor.matmul(
                            raw[0:HD + 1, nh * 512:(nh + 1) * 512],
                            k_aug[:, mb, h * (HD + 1):(h + 1) * (HD + 1)],
                            E[:, mb, hh * N + nh * 512: hh * N + (nh + 1) * 512],
                            start=(mb == 0), stop=(mb == MB - 1),
                        )
                # psum->sbuf copies: rows 0..63 feed the normalize TT; row 64
                # (= r) lands on partition 0 via a PSUM-read partition shift
                # (DVE can read PSUM rows from any lane; custom-DVE recip and
                # GpSimd broadcast both need absolute partition-0 SBUF).
                raw_sb = attn.tile([HD, N], f32, tag="raw_sb", bufs=2, name=f"raw_sb{p}_{hh}")
                nc.vector.tensor_copy(out=raw_sb[:], in_=raw[0:HD, :])
                r0 = attn.tile([1, N], f32, tag="r_row", bufs=2, name=f"r0{p}_{hh}")
                nc.vector.tensor_copy(out=r0[:], in_=raw[HD:HD + 1, :])
                raws.append((raw_sb, r0))
            for hh in range(2):
                base = hh * 64
                raw_sb, r0 = raws[hh]
                rinv = attn.tile([1, N], f32, tag="r_rec", bufs=2, name=f"rinv{p}_{hh}")
                nc.vector.reciprocal_approx_fast(out=rinv[:], in_=r0[:])
                rb = attn.tile([HD, N], f32, tag="bc", bufs=2, name=f"rb{p}_{hh}")
                nc.gpsimd.partition_broadcast(rb[:], rinv[:], channels=HD)
                nc.vector.tensor_tensor(
                    attn_sb[base:base + 64, :], raw_sb[0:HD, :], rb[:], MUL
                )
                if DEBUG_DUMP and p == 0 and hh == 0:
                    nc.sync.dma_start(dbg["dbg_rawsb0"][0:HD], raw_sb[:])
                    nc.sync.dma_start(dbg["dbg_rawsb0"][HD:HD + 1], r0[:])
                    nc.sync.dma_start(dbg["dbg_rinvb0"][:], rb[:])

            # second softmax (over head_dim) + z.T
            y = attn.tile([128, N], bf16, tag="y", bufs=2, name=f"y{p}")
            nc.vector.tensor_tensor(y[:], attn_sb[:], lip_vT[:, p, :], MUL)
            expY = attn.tile([128, N], bf16, tag="expY", bufs=2, name=f"expY{p}")
            nc.scalar.activation(expY[:], y[:], AF.Exp)

            s_ps = psB.tile([128, N], f32, tag="psB")
            for nh in range(NH):
                nc.tensor.matmul(
                    s_ps[0:2, nh * 512:(nh + 1) * 512],
                    sel_sb[:],
                    expY[:, nh * 512:(nh + 1) * 512],
                    start=True, stop=True,
                )
            s_sb = attn.tile([2, N], f32, tag="raw_sb", bufs=2, name=f"s_sb{p}")
            nc.vector.tensor_copy(out=s_sb[:], in_=s_ps[0:2, :])
            s_rec = attn.tile([2, N], f32, tag="r_rec", bufs=2, name=f"srec{p}")
            nc.vector.reciprocal_approx_fast(out=s_rec[:], in_=s_sb[:])
            s_bf = attn.tile([2, N], bf16, tag="r_row", bufs=2, name=f"sbf{p}")
            nc.vector.tensor_copy(out=s_bf[:], in_=s_rec[:])
            # broadcast 1/s across partitions with K=1 bf16 matmuls into PSUM
            # (PSUM TT operands are partition-alignment-exempt)
            sb_lo = psB.tile([128, N], f32, tag="psB")
            sb_hi = psB.tile([128, N], f32, tag="psB")
            for nh in range(NH):
                nc.tensor.matmul(
                    sb_lo[0:64, nh * 512:(nh + 1) * 512], selB[:, 0:64],
                    s_bf[:, nh * 512:(nh + 1) * 512], start=True, stop=True,
                )
                nc.tensor.matmul(
                    sb_hi[0:64, nh * 512:(nh + 1) * 512], selB[:, 64:128],
                    s_bf[:, nh * 512:(nh + 1) * 512], start=True, stop=True,
                )

            u = attn.tile([128, N], bf16, tag="y", bufs=2, name=f"u{p}")
            nc.vector.tensor_tensor(u[:], attn_sb[:], expY[:], MUL)
            nc.vector.tensor_tensor(zT[0:64, p, :], u[0:64, :], sb_lo[0:64, :], MUL)
            nc.vector.tensor_tensor(zT[64:128, p, :], u[64:128, :], sb_hi[0:64, :], MUL)
            if DEBUG_DUMP and p == 0:
                nc.sync.dma_start(dbg["dbg_attn0"][:], attn_sb[:])
                nc.sync.dma_start(dbg["dbg_y0"][:], y[:])
                nc.sync.dma_start(dbg["dbg_expY0"][:], expY[:])
                nc.sync.dma_start(dbg["dbg_srec0"][:], s_rec[:])

        # 2-deep software pipeline: S(p+1) overlaps qk(p); the independent
        # projection/SE matmuls are spread through the ACT-bound S windows
        # as PE filler.  Wc lands after qk(3)'s raws so its matmuls fill the
        # DVE-bound normalize tail before the final output GEMM.
        emit_qkT()
        E0 = emit_S(0)
        emit_kaug()
        E1 = emit_S(1)
        emit_lipv()
        emit_qk(0, E0)
        if DEBUG_DUMP:
            nc.sync.dma_start(dbg["dbg_E0"][:], E0[:])
        E2 = emit_S(2)
        emit_qs()
        emit_qk(1, E1)
        E3 = emit_S(3)
        emit_qk(2, E2)
        emit_se()
        emit_qk(3, E3)
        emit_wc()
        attn_ctx.__exit__(None, None, None)
        if DEBUG_DUMP:
            nc.sync.dma_start(dbg["dbg_qT"][:], qT[:])
            nc.sync.dma_start(dbg["dbg_kT"][:], kT[:])
            nc.sync.dma_start(dbg["dbg_kaug"][:], k_aug[:])
            nc.sync.dma_start(dbg["dbg_lipv"][:], lip_vT[:])
            nc.sync.dma_start(dbg["dbg_qs"][:], qs[:])
            nc.sync.dma_start(dbg["dbg_zT"][:], zT[:])
            nc.sync.dma_start(dbg["dbg_avattn"][:], av_attn[:])

        # ---- fused output GEMM: out[n,d] = sum_c z[n,c] Wc[c,d] + row[d] ----
        with tc.tile_pool(name="outp", bufs=2) as outp:
            for nb in range(MB):
                ps = psA.tile([128, N], f32, tag="psA")
                for cb in range(CB):
                    nc.tensor.matmul(
                        ps[:, 0:512],
                        zT[:, cb, nb * 128:(nb + 1) * 128],
                        Wc_sb[:, cb, :],
                        start=(cb == 0), stop=(cb == CB - 1),
                    )
                o_sb = outp.tile([128, C], f32, tag="o_sb")
                nc.vector.tensor_tensor(o_sb[:], ps[:, 0:512], rowb[:], ADD)
                nc.sync.dma_start(
                    d_out[:].rearrange("(nb p) c -> nb p c", p=128)[nb], o_sb[:]
                )

    nc.compile()
    return nc


def _marshal(audia, lip, W_qkv, W_lip, W_proj, b_proj, W_se):
    bf16 = ml_dtypes.bfloat16
    WqkT = np.ascontiguousarray(W_qkv[:2 * C].T.astype(bf16))
    WlipT = np.ascontiguousarray(W_lip.T.astype(bf16))
    WseT = np.ascontiguousarray(W_se.T.astype(bf16))
    WprojN = np.ascontiguousarray(W_proj.astype(bf16))
    bprojB = np.ascontiguousarray(
        np.asarray(b_proj, np.float32).reshape(CB, 128).T.astype(bf16)
    )
    sel = np.zeros((128, 2), bf16)
    sel[0:64, 0] = 1
    sel[64:128, 1] = 1
    selB = np.zeros((2, 128), bf16)
    selB[0, 0:64] = 1
    selB[1, 64:128] = 1
    in_maps = []
    for b in range(B):
        in_maps.append({
            "audiaT": np.ascontiguousarray(audia[b].T.astype(bf16)),
            "lipT": np.ascontiguousarray(lip[b].T.astype(bf16)),
            "WqkT": WqkT, "WlipT": WlipT, "WseT": WseT, "WprojN": WprojN,
            "bprojB": bprojB, "sel": sel, "selB": selB,
        })
    return in_maps


def run(inputs, trace=False, **kw):
    from concourse.bass_utils import run_bass_kernel_spmd
    if "nc" not in _CACHED:
        _CACHED["nc"] = build_nc()
    in_maps = _marshal(**inputs)
    return run_bass_kernel_spmd(
        _CACHED["nc"], in_maps, core_ids=list(range(B)), trace=trace, **kw
    )


def kernel(audia, lip, W_qkv, W_lip, W_proj, b_proj, W_se):
    res = run(dict(audia=audia, lip=lip, W_qkv=W_qkv, W_lip=W_lip,
                   W_proj=W_proj, b_proj=b_proj, W_se=W_se))
    return np.stack([r["out"] for r in res.results], 0).astype(np.float32)


# revision 24
# speedup vs baseline: 1.2760x; 1.0223x over previous
"""Trainium2 Bass kernel for nn_AV_MiltiHeadAttention.

Strategy: data-parallel over B across 8 NeuronCores (1 batch element per core).
Per-core everything is kept in a "transposed" (feature-on-partitions) layout so
no on-chip transposes are needed:

  audiaT/lipT/W*T are marshalled on host (transpose + bf16 cast = input
  layout prep for the SPMD shards), all matmul contractions then have their
  contraction dim on SBUF partitions naturally.

  S.T[m,n] = sum_d kT[d,m] qT[d,n]          (per head, K=64, 2 heads row-packed)
  E = exp(S.T * scale)                       (no max subtraction; |S*scale|<2)
  raw[j,n] = sum_m [1 | k_h][m,j] E[m,n]    -> row 0 = softmax denominator r,
                                               rows 1..64 = attn_qk.T (unnorm)
  rinv = 1/r (DVE recip on [1,N]); broadcast across 64 partitions on GpSimd
  attn_qk.T = raw[1:65] * rinv_b
  y = attn_qk.T * lip_v.T ; expY = exp(y)
  s = colsum_d(expY) via 2-col selector matmul (softmax over head_dim)
  z.T = attn_qk.T * expY * (1/s)             (1/s broadcast on GpSimd)
  av_attn = sigmoid(2 * qs.T@qs / temp)      (SE layer)
  Wc = W_proj.T @ av_attn ; row = b_proj @ av_attn   (proj/out fusion)
  out = z @ Wc + row                          (single fused output GEMM)

All matmuls bf16 x bf16 -> fp32 PSUM. Softmax normalization in fp32.
"""

import os
import numpy as np
import ml_dtypes

DEBUG_DUMP = os.environ.get("KDBG", "0") == "1"

B, N, C = 8, 1024, 512
H, HD = 8, 64
CB = C // 128          # 4 chunks of the feature dim
MB = N // 128          # 8 chunks of the token dim
NH = N // 512          # 2 halves of the token dim (matmul free dim = 512)
SCALE = HD ** -0.5
TEMP = C ** 0.5

_CACHED = {}


def build_nc():
    import concourse.bass as bass
    import concourse.tile as tile
    import concourse.mybir as mybir
    from concourse import bacc
    from concourse import library_config
    from contextlib import ExitStack

    f32 = mybir.dt.float32
    bf16 = mybir.dt.bfloat16
    AF = mybir.ActivationFunctionType
    MUL = mybir.AluOpType.mult
    ADD = mybir.AluOpType.add

    nc = bacc.Bacc("TRN2", target_bir_lowering=False, debug=False, num_devices=B)

    d_audiaT = nc.dram_tensor("audiaT", [C, N], bf16, kind="ExternalInput")
    d_lipT = nc.dram_tensor("lipT", [C, N], bf16, kind="ExternalInput")
    d_WqkT = nc.dram_tensor("WqkT", [C, 2 * C], bf16, kind="ExternalInput")
    d_WlipT = nc.dram_tensor("WlipT", [C, C], bf16, kind="ExternalInput")
    d_WseT = nc.dram_tensor("WseT", [C, C], bf16, kind="ExternalInput")
    d_WprojN = nc.dram_tensor("WprojN", [C, C], bf16, kind="ExternalInput")
    d_bprojB = nc.dram_tensor("bprojB", [128, CB], bf16, kind="ExternalInput")
    d_sel = nc.dram_tensor("sel", [128, 2], bf16, kind="ExternalInput")
    d_selB = nc.dram_tensor("selB", [2, 128], bf16, kind="ExternalInput")
    d_out = nc.dram_tensor("out", [N, C], f32, kind="ExternalOutput")
    dbg = {}
    if DEBUG_DUMP:
        for nm, shp, dt in [
            ("dbg_qT", [128, CB, N], bf16), ("dbg_kT", [128, CB, N], bf16),
            ("dbg_kaug", [128, MB, H * (HD + 1)], bf16),
            ("dbg_lipv", [128, CB, N], bf16), ("dbg_qs", [128, MB, C], bf16),
            ("dbg_E0", [128, MB, 2 * N], bf16),
            ("dbg_rawsb0", [HD + 1, N], f32), ("dbg_attn0", [128, N], bf16),
            ("dbg_rinvb0", [HD, N], f32),
            ("dbg_y0", [128, N], bf16), ("dbg_expY0", [128, N], bf16),
            ("dbg_srec0", [2, N], f32),
            ("dbg_sblo0", [HD, N], f32), ("dbg_sbhi0", [HD, N], f32),
            ("dbg_zT", [128, CB, N], bf16), ("dbg_avattn", [128, CB, C], bf16),
            ("dbg_wc", [128, CB, C], bf16), ("dbg_rowb", [128, C], f32),
        ]:
            dbg[nm] = nc.dram_tensor(nm, shp, dt, kind="ExternalOutput")

    with tile.TileContext(nc) as tc, ExitStack() as ctx:
        persist = ctx.enter_context(tc.tile_pool(name="persist", bufs=1))
        psA = ctx.enter_context(tc.tile_pool(name="psA", bufs=2, space="PSUM"))
        psB = ctx.enter_context(tc.tile_pool(name="psB", bufs=2, space="PSUM"))

        nc.gpsimd.load_library(library_config.attn)

        # ---- persistent SBUF tensors ----
        qT = persist.tile([128, CB, N], bf16, tag="qT")        # q.T  [(h,d), n]
        kT = persist.tile([128, CB, N], bf16, tag="kT")        # k.T  [(h,d), n]
        k_aug = persist.tile([128, MB, H * (HD + 1)], bf16, tag="k_aug")
        lip_vT = persist.tile([128, CB, N], bf16, tag="lip_vT")
        qs = persist.tile([128, MB, C], bf16, tag="qs")        # qs natural [n, c]
        WprojN_sb = persist.tile([128, CB, C], bf16, tag="WprojN_sb")
        Wc_sb = persist.tile([128, CB, C], bf16, tag="Wc_sb")
        bprojB_sb = persist.tile([128, CB], bf16, tag="bprojB_sb")
        sel_sb = persist.tile([128, 2], bf16, tag="sel_sb")
        zT = persist.tile([128, CB, N], bf16, tag="zT")
        av_attn = persist.tile([128, CB, C], bf16, tag="av_attn")
        rowb = persist.tile([128, C], f32, tag="rowb")
        # K=2 selector stationaries for the 1/s partition broadcast matmuls:
        # selB[:, 0:64] picks row 0 (s_lo), selB[:, 64:128] picks row 1 (s_hi)
        selB = persist.tile([2, 128], bf16, tag="selB")

        early = ctx.enter_context(tc.tile_pool(name="early", bufs=1))
        audiaT_sb = early.tile([128, CB, N], bf16, tag="audiaT_sb")
        lipT_sb = early.tile([128, CB, N], bf16, tag="lipT_sb")
        WqkT_sb = early.tile([128, CB, 2 * C], bf16, tag="WqkT_sb")
        WlipT_sb = early.tile([128, CB, C], bf16, tag="WlipT_sb")
        WseT_sb = early.tile([128, CB, C], bf16, tag="WseT_sb")

        # input DMAs ordered by first use: qkT needs WqkT(q half) + audiaT
        # first; the k half feeds emit_kaug; lip/se/proj weights follow.
        d_WqkT_r = d_WqkT[:].rearrange("(cb p) c -> cb p c", p=128)
        d_audiaT_r = d_audiaT[:].rearrange("(cb p) n -> cb p n", p=128)
        for cb in range(CB):
            nc.sync.dma_start(WqkT_sb[:, cb, 0:C], d_WqkT_r[cb][:, 0:C])
            nc.sync.dma_start(audiaT_sb[:, cb], d_audiaT_r[cb])
        for cb in range(CB):
            nc.sync.dma_start(WqkT_sb[:, cb, C:2 * C], d_WqkT_r[cb][:, C:2 * C])
        nc.sync.dma_start(WlipT_sb[:], d_WlipT[:].rearrange("(cb p) c -> p cb c", p=128))
        nc.sync.dma_start(lipT_sb[:], d_lipT[:].rearrange("(cb p) n -> p cb n", p=128))
        nc.sync.dma_start(sel_sb[:], d_sel[:])
        nc.sync.dma_start(selB[:], d_selB[:])
        nc.sync.dma_start(WseT_sb[:], d_WseT[:].rearrange("(cb p) c -> p cb c", p=128))
        nc.sync.dma_start(WprojN_sb[:], d_WprojN[:].rearrange("(cb p) c -> p cb c", p=128))
        nc.sync.dma_start(bprojB_sb[:], d_bprojB[:])

        # ---- P1 pieces (emitted interleaved with attention below) ----
        def emit_qkT():
            # qT / kT: out [oc:128, n] ; lhsT = WqkT block, rhs = audiaT
            for dst, oc0 in ((qT, 0), (kT, C)):
                for ocb in range(CB):
                    ps = psA.tile([128, N], f32, tag="psA")
                    for cb in range(CB):
                        for nh in range(NH):
                            nc.tensor.matmul(
                                ps[:, nh * 512:(nh + 1) * 512],
                                WqkT_sb[:, cb, oc0 + ocb * 128: oc0 + (ocb + 1) * 128],
                                audiaT_sb[:, cb, nh * 512:(nh + 1) * 512],
                                start=(cb == 0), stop=(cb == CB - 1),
                            )
                    nc.vector.tensor_copy(out=dst[:, ocb, :], in_=ps[:])

        def emit_kaug():
            # k natural (+ trailing ones column: raw row 64 = softmax denom r)
            for mb in range(MB):
                ps = psB.tile([128, N], f32, tag="psB")
                for cb in range(CB):
                    nc.tensor.matmul(
                        ps[:, 0:512],
                        audiaT_sb[:, cb, mb * 128:(mb + 1) * 128],
                        WqkT_sb[:, cb, C:2 * C],
                        start=(cb == 0), stop=(cb == CB - 1),
                    )
                nc.vector.tensor_copy(
                    out=k_aug[:, mb].rearrange("p (h e) -> p h e", e=HD + 1)[:, :, 0:HD],
                    in_=ps[:, 0:512].rearrange("p (h d) -> p h d", d=HD),
                )
            nc.vector.memset(
                k_aug[:].rearrange("p m (h e) -> p m h e", e=HD + 1)[:, :, :, HD:HD + 1], 1.0
            )

        def emit_lipv():
            for ocb in range(CB):
                ps = psB.tile([128, N], f32, tag="psB")
                for cb in range(CB):
                    for nh in range(NH):
                        nc.tensor.matmul(
                            ps[:, nh * 512:(nh + 1) * 512],
                            WlipT_sb[:, cb, ocb * 128:(ocb + 1) * 128],
                            lipT_sb[:, cb, nh * 512:(nh + 1) * 512],
                            start=(cb == 0), stop=(cb == CB - 1),
                        )
                nc.vector.tensor_copy(out=lip_vT[:, ocb, :], in_=ps[:])

        def emit_qs():
            # qs natural [n, c] (needed on both sides of the SE bmm)
            for nb in range(MB):
                ps = psB.tile([128, N], f32, tag="psB")
                for cb in range(CB):
                    nc.tensor.matmul(
                        ps[:, 0:512],
                        audiaT_sb[:, cb, nb * 128:(nb + 1) * 128],
                        WseT_sb[:, cb, :],
                        start=(cb == 0), stop=(cb == CB - 1),
                    )
                nc.vector.tensor_copy(out=qs[:, nb, :], in_=ps[:, 0:512])

        # ---- attention + SE ----
        attn_ctx = tc.tile_pool(name="attn", bufs=1)
        attn = attn_ctx.__enter__()

        def emit_se():
            # x[c,d] = sum_n qs[n,c] qs[n,d];  av_attn = sigmoid(2x/temp)
            for cb in range(CB):
                ps = psB.tile([128, N], f32, tag="psB")
                for nb in range(MB):
                    nc.tensor.matmul(
                        ps[:, 0:512],
                        qs[:, nb, cb * 128:(cb + 1) * 128],
                        qs[:, nb, :],
                        start=(nb == 0), stop=(nb == MB - 1),
                    )
                nc.scalar.activation(av_attn[:, cb, :], ps[:, 0:512], AF.Sigmoid,
                                     scale=2.0 / TEMP)

        def emit_wc():
            # Wc = W_proj.T @ av_attn  (fold the proj GEMM into the output GEMM)
            for ccb in range(CB):
                ps = psA.tile([128, N], f32, tag="psA")
                for eb in range(CB):
                    nc.tensor.matmul(
                        ps[:, 0:512],
                        WprojN_sb[:, eb, ccb * 128:(ccb + 1) * 128],
                        av_attn[:, eb, :],
                        start=(eb == 0), stop=(eb == CB - 1),
                    )
                nc.scalar.activation(Wc_sb[:, ccb, :], ps[:, 0:512], AF.Copy)
            # row = b_proj @ av_attn, broadcast across partitions for the
            # final bias add (fused into the PSUM->SBUF evacuation).
            rp = psB.tile([128, N], f32, tag="psB")
            for cb in range(CB):
                nc.tensor.matmul(
                    rp[0:1, 0:512],
                    bprojB_sb[:, cb:cb + 1],
                    av_attn[:, cb, :],
                    start=(cb == 0), stop=(cb == CB - 1),
                )
            row_sb = persist.tile([1, C], f32, tag="row_sb")
            nc.vector.tensor_copy(out=row_sb[:], in_=rp[0:1, 0:512])
            nc.gpsimd.partition_broadcast(rowb[:], row_sb[:], channels=128)
            if DEBUG_DUMP:
                nc.sync.dma_start(dbg["dbg_wc"][:], Wc_sb[:])
                nc.sync.dma_start(dbg["dbg_rowb"][:], rowb[:])

        def emit_S(p):
            # E = exp(S.T * scale) for heads (2p, 2p+1)
            E = attn.tile([128, MB, 2 * N], bf16, tag="E", bufs=2, name=f"E{p}")
            for mb in range(MB):
                for hh in range(2):
                    ps = psA.tile([128, N], f32, tag="psA")
                    base = hh * 64
                    for nh in range(NH):
                        nc.tensor.matmul(
                            ps[:, nh * 512:(nh + 1) * 512],
                            kT[base:base + 64, p, mb * 128:(mb + 1) * 128],
                            qT[base:base + 64, p, nh * 512:(nh + 1) * 512],
                            start=True, stop=True,
                            tile_position=(base, 0),
                        )
                    nc.scalar.activation(
                        E[:, mb, hh * N:(hh + 1) * N], ps[:], AF.Exp, scale=SCALE
                    )
            return E

        def emit_qk(p, E):
            # raw = [1|k_h].T @ E_h for BOTH heads back-to-back (keeps the PE
            # dense so HAM stays at full clock), then the normalize chains.
            attn_sb = attn.tile([128, N], bf16, tag="attn_sb", bufs=2, name=f"attn_sb{p}")
            raws = []
            for hh in range(2):
                h = 2 * p + hh
                raw = psB.tile([128, N], f32, tag="psB")
                for mb in range(MB):
                    for nh in range(NH):
                        nc.tensor.matmul(
                            raw[0:HD + 1, nh * 512:(nh + 1) * 512],
                            k_aug[:, mb, h * (HD + 1):(h + 1) * (HD + 1)],
                            E[:, mb, hh * N + nh * 512: hh * N + (nh + 1) * 512],
                            start=(mb == 0), stop=(mb == MB - 1),
                        )
                # psum->sbuf copies: rows 0..63 feed the normalize TT; row 64
                # (= r) lands on partition 0 via a PSUM-read partition shift
                # (DVE can read PSUM rows from any lane; custom-DVE recip and
                # GpSimd broadcast both need absolute partition-0 SBUF).
                raw_sb = attn.tile([HD, N], f32, tag="raw_sb", bufs=2, name=f"raw_sb{p}_{hh}")
                nc.vector.tensor_copy(out=raw_sb[:], in_=raw[0:HD, :])
                r0 = attn.tile([1, N], f32, tag="r_row", bufs=2, name=f"r0{p}_{hh}")
                nc.vector.tensor_copy(out=r0[:], in_=raw[HD:HD + 1, :])
                raws.append((raw_sb, r0))
            for hh in range(2):
                base = hh * 64
                raw_sb, r0 = raws[hh]
                rinv = attn.tile([1, N], f32, tag="r_rec", bufs=2, name=f"rinv{p}_{hh}")
                nc.vector.reciprocal_approx_fast(out=rinv[:], in_=r0[:])
                rb = attn.tile([HD, N], f32, tag="bc", bufs=2, name=f"rb{p}_{hh}")
                nc.gpsimd.partition_broadcast(rb[:], rinv[:], channels=HD)
                nc.vector.tensor_tensor(
                    attn_sb[base:base + 64, :], raw_sb[0:HD, :], rb[:], MUL
                )
                if DEBUG_DUMP and p == 0 and hh == 0:
                    nc.sync.dma_start(dbg["dbg_rawsb0"][0:HD], raw_sb[:])
                    nc.sync.dma_start(dbg["dbg_rawsb0"][HD:HD + 1], r0[:])
                    nc.sync.dma_start(dbg["dbg_rinvb0"][:], rb[:])

            # second softmax (over head_dim) + z.T
            y = attn.tile([128, N], bf16, tag="y", bufs=2, name=f"y{p}")
            nc.vector.tensor_tensor(y[:], attn_sb[:], lip_vT[:, p, :], MUL)
            expY = attn.tile([128, N], bf16, tag="expY", bufs=2, name=f"expY{p}")
            nc.scalar.activation(expY[:], y[:], AF.Exp)

            s_ps = psB.tile([128, N], f32, tag="psB")
            for nh in range(NH):
                nc.tensor.matmul(
                    s_ps[0:2, nh * 512:(nh + 1) * 512],
                    sel_sb[:],
                    expY[:, nh * 512:(nh + 1) * 512],
                    start=True, stop=True,
                )
            s_sb = attn.tile([2, N], f32, tag="raw_sb", bufs=2, name=f"s_sb{p}")
            nc.vector.tensor_copy(out=s_sb[:], in_=s_ps[0:2, :])
            s_rec = attn.tile([2, N], f32, tag="r_rec", bufs=2, name=f"srec{p}")
            nc.vector.reciprocal_approx_fast(out=s_rec[:], in_=s_sb[:])
            s_bf = attn.tile([2, N], bf16, tag="r_row", bufs=2, name=f"sbf{p}")
            nc.vector.tensor_copy(out=s_bf[:], in_=s_rec[:])
            # broadcast 1/s across partitions with K=2 bf16 matmuls into PSUM
            # (PSUM TT operands are partition-alignment-exempt).  For p==2 the
            # S matmuls are done with psA, so park these there — they hold
            # their buffer until the end of the normalize chain and would
            # otherwise starve the psB ring that the se/wc fillers need.  (p==3
            # must stay on psB: psA holds the partial output accumulators.)
            sp = psA if p == 2 else psB
            sb_lo = sp.tile([128, N], f32, tag="psA" if p == 2 else "psB")
            sb_hi = sp.tile([128, N], f32, tag="psA" if p == 2 else "psB")
            for nh in range(NH):
                nc.tensor.matmul(
                    sb_lo[0:64, nh * 512:(nh + 1) * 512], selB[:, 0:64],
                    s_bf[:, nh * 512:(nh + 1) * 512], start=True, stop=True,
                )
                nc.tensor.matmul(
                    sb_hi[0:64, nh * 512:(nh + 1) * 512], selB[:, 64:128],
                    s_bf[:, nh * 512:(nh + 1) * 512], start=True, stop=True,
                )

            u = attn.tile([128, N], bf16, tag="y", bufs=2, name=f"u{p}")
            nc.vector.tensor_tensor(u[:], attn_sb[:], expY[:], MUL)
            nc.vector.tensor_tensor(zT[0:64, p, :], u[0:64, :], sb_lo[0:64, :], MUL)
            nc.vector.tensor_tensor(zT[64:128, p, :], u[64:128, :], sb_hi[0:64, :], MUL)
            if DEBUG_DUMP and p == 0:
                nc.sync.dma_start(dbg["dbg_attn0"][:], attn_sb[:])
                nc.sync.dma_start(dbg["dbg_y0"][:], y[:])
                nc.sync.dma_start(dbg["dbg_expY0"][:], expY[:])
                nc.sync.dma_start(dbg["dbg_srec0"][:], s_rec[:])

        # 2-deep software pipeline: S(p+1) overlaps qk(p); the independent
        # projection/SE matmuls are spread through the ACT-bound S windows
        # as PE filler.  se/wc and the cb=0..2 partial output accumulation
        # fill the DVE-bound qk(2)/qk(3) normalize windows; only the cb=3
        # matmul + bias + DMA remain after zT(3) lands.
        emit_qkT()
        E0 = emit_S(0)
        emit_kaug()
        E1 = emit_S(1)
        emit_lipv()
        emit_qk(0, E0)
        if DEBUG_DUMP:
            nc.sync.dma_start(dbg["dbg_E0"][:], E0[:])
        E2 = emit_S(2)
        emit_qs()
        emit_qk(1, E1)
        E3 = emit_S(3)
        emit_qk(2, E2)
        emit_se()
        emit_wc()
        # partial output accumulation for nb 0..3 (zT chunks 0..2 are ready);
        # two nb chunks pack into each held [128,1024] psA tile.
        parts = []
        for i in range(2):
            ps = psA.tile([128, N], f32, tag="psA")
            for half in range(2):
                nb = 2 * i + half
                for cb in range(CB - 1):
                    nc.tensor.matmul(
                        ps[:, half * 512:(half + 1) * 512],
                        zT[:, cb, nb * 128:(nb + 1) * 128],
                        Wc_sb[:, cb, :],
                        start=(cb == 0), stop=False,
                    )
            parts.append(ps)
        emit_qk(3, E3)
        attn_ctx.__exit__(None, None, None)
        if DEBUG_DUMP:
            nc.sync.dma_start(dbg["dbg_qT"][:], qT[:])
            nc.sync.dma_start(dbg["dbg_kT"][:], kT[:])
            nc.sync.dma_start(dbg["dbg_kaug"][:], k_aug[:])
            nc.sync.dma_start(dbg["dbg_lipv"][:], lip_vT[:])
            nc.sync.dma_start(dbg["dbg_qs"][:], qs[:])
            nc.sync.dma_start(dbg["dbg_zT"][:], zT[:])
            nc.sync.dma_start(dbg["dbg_avattn"][:], av_attn[:])

        # ---- fused output GEMM: out[n,d] = sum_c z[n,c] Wc[c,d] + row[d] ----
        d_out_r = d_out[:].rearrange("(nb p) c -> nb p c", p=128)
        with tc.tile_pool(name="outp", bufs=3) as outp:
            # nb 0..3: finish the held partials (cb=3 only)
            for i in range(2):
                ps = parts[i]
                for half in range(2):
                    nb = 2 * i + half
                    nc.tensor.matmul(
                        ps[:, half * 512:(half + 1) * 512],
                        zT[:, CB - 1, nb * 128:(nb + 1) * 128],
                        Wc_sb[:, CB - 1, :],
                        start=False, stop=True,
                    )
                for half in range(2):
                    nb = 2 * i + half
                    o_sb = outp.tile([128, C], f32, tag="o_sb")
                    nc.vector.tensor_tensor(
                        o_sb[:], ps[:, half * 512:(half + 1) * 512], rowb[:], ADD
                    )
                    nc.sync.dma_start(d_out_r[nb], o_sb[:])
            # nb 4..7: full accumulation chains through the freed psA ring
            for i in range(2):
                ps = psA.tile([128, N], f32, tag="psA")
                for half in range(2):
                    nb = 4 + 2 * i + half
                    for cb in range(CB):
                        nc.tensor.matmul(
                            ps[:, half * 512:(half + 1) * 512],
                            zT[:, cb, nb * 128:(nb + 1) * 128],
                            Wc_sb[:, cb, :],
                            start=(cb == 0), stop=(cb == CB - 1),
                        )
                for half in range(2):
                    nb = 4 + 2 * i + half
                    o_sb = outp.tile([128, C], f32, tag="o_sb")
                    nc.vector.tensor_tensor(
                        o_sb[:], ps[:, half * 512:(half + 1) * 512], rowb[:], ADD
                    )
                    nc.sync.dma_start(d_out_r[nb], o_sb[:])

    nc.compile()
    return nc


def _marshal(audia, lip, W_qkv, W_lip, W_proj, b_proj, W_se):
    bf16 = ml_dtypes.bfloat16
    WqkT = np.ascontiguousarray(W_qkv[:2 * C].T.astype(bf16))
    WlipT = np.ascontiguousarray(W_lip.T.astype(bf16))
    WseT = np.ascontiguousarray(W_se.T.astype(bf16))
    WprojN = np.ascontiguousarray(W_proj.astype(bf16))
    bprojB = np.ascontiguousarray(
        np.asarray(b_proj, np.float32).reshape(CB, 128).T.astype(bf16)
    )
    sel = np.zeros((128, 2), bf16)
    sel[0:64, 0] = 1
    sel[64:128, 1] = 1
    selB = np.zeros((2, 128), bf16)
    selB[0, 0:64] = 1
    selB[1, 64:128] = 1
    in_maps = []
    for b in range(B):
        in_maps.append({
            "audiaT": np.ascontiguousarray(audia[b].T.astype(bf16)),
            "lipT": np.ascontiguousarray(lip[b].T.astype(bf16)),
            "WqkT": WqkT, "WlipT": WlipT, "WseT": WseT, "WprojN": WprojN,
            "bprojB": bprojB, "sel": sel, "selB": selB,
        })
    return in_maps


def run(inputs, trace=False, **kw):
    from concourse.bass_utils import run_bass_kernel_spmd
    if "nc" not in _CACHED:
        _CACHED["nc"] = build_nc()
    in_maps = _marshal(**inputs)
    return run_bass_kernel_spmd(
        _CACHED["nc"], in_maps, core_ids=list(range(B)), trace=trace, **kw
    )


def kernel(audia, lip, W_qkv, W_lip, W_proj, b_proj, W_se):
    res = run(dict(audia=audia, lip=lip, W_qkv=W_qkv, W_lip=W_lip,
                   W_proj=W_proj, b_proj=b_proj, W_se=W_se))
    return np.stack([r["out"] for r in res.results], 0).astype(np.float32)
